# revision 60
# baseline (speedup 1.0000x reference)
"""Trainium2 Bass kernel for a 2-layer Kipf GCN (nn_KipfGCN_1743756722177).

Strategy (8 NeuronCores, SPMD):
  - Nodes sharded contiguously across cores (12500 each, padded to 12544).
    Edges (incl. self-loops) partitioned by destination core.
  - Symmetric normalization dinv[src]*dinv[dst] is folded into table row
    scaling (src side) and per-partition output scaling (dst side); no
    per-edge multiplies.
  - Per layer, a compact bf16 node-feature table ([N,64] rows) is computed
    shard-wise and AllGathered so every core can gather any row.
  - Messages are fetched with dma_gather (int16 indices) at PAIR
    granularity: one 256-byte gather row holds two consecutive nodes'
    features (64+64 bf16). The pair parity is resolved during aggregation
    by two parity-masked one-hot matrices, so no bytes are wasted and the
    AllGather moves half the data of an fp32 layout.
  - Edge slots are grouped by (tile-block, window, dest-tile), sorted by
    source row within each bucket (HBM locality), padded to 128-slot
    chunks (pad slots gather row 0 and are masked out of the aggregation).
    The node->tile assignment is balanced host-side (2D bin packing of
    per-window in-degrees) so every bucket packs just under a multiple of
    128 on all 8 cores: slot padding drops from 25% to 1.4%. This matters
    because dma_gather descriptor generation runs serially on the gpsimd
    engine at ~4ns/slot — the kernel's bottleneck. Gather calls
    round-robin over 4 SWDGE queues (per-queue rings / Q7 core pairs).
  - The per-layer AllGather is split in two half-shard collectives issued
    as soon as each half of the table is written, overlapping them with
    phase-A compute (layer 1) and the previous layer's gather stream
    (layer 2); the halves coincide with the two int16 index windows.
  - Aggregation per 128-node destination tile: for each 128-edge chunk,
    bf16 one-hot matrices S_e/S_o (S[e,j] = dst_rel[e]==j, masked to
    even/odd-parity slots, built by the vector engine from an iota
    constant) are contracted on the tensor engine with fp32 PSUM
    accumulation against the corresponding half of the gathered pair row.
  - All floating-point math runs on device; the host only does integer
    index prep, sharding/layout, and de-padding of the output.
"""

import os

import numpy as np

import concourse.bass as bass
import concourse.bacc as bacc
import concourse.mybir as mybir
import concourse.tile as tile
from concourse import library_config
from concourse.bass_utils import run_bass_kernel_spmd

P = 128
NCORE = 8
TBS = 3                       # dest tiles per tile-block
NWIN = 2                      # int16 index windows over the pair table
GMAX = 1024                   # max indices per dma_gather call (HW limit)
SCRATCH = 32768               # SWDGE ring bytes/partition (2048 descs)
NQ = 4                        # SWDGE queues; gather calls round-robin
F32 = mybir.dt.float32
BF16 = mybir.dt.bfloat16
I16 = mybir.dt.int16
NPBF16 = mybir.dt.np(mybir.dt.bfloat16)
ROWW = 128                    # gather row width: 2 nodes x 64 feats bf16


# ---------------------------------------------------------------------------
# Host-side preprocessing
# ---------------------------------------------------------------------------

def _balance(d0, d1, ntile, caps):
    """Greedy 2D bin packing: assign nodes (with per-window in-degrees
    d0/d1) to ntile tiles of 128 nodes so both per-window bucket sums
    stay at/under caps[t] (multiples of 128). Minimizes gather padding."""
    nn = len(d0)
    order = np.argsort(-(d0 + d1), kind="stable")
    cnt = np.zeros(ntile, dtype=np.int64)
    s0 = np.zeros(ntile, dtype=np.int64)
    s1 = np.zeros(ntile, dtype=np.int64)
    assign = np.empty(nn, dtype=np.int64)
    # proportional fill targets: bins fill toward their cap share, so
    # larger (overflow) bins genuinely absorb the over-mean excess
    t0 = caps * (max(int(d0.sum()), 1) / caps.sum())
    t1 = caps * (max(int(d1.sum()), 1) / caps.sum())
    for n in order:
        a, b = d0[n], d1[n]
        over = (np.maximum(s0 + a - caps, 0) + np.maximum(s1 + b - caps, 0))
        fill = np.maximum((s0 + a) / t0, (s1 + b) / t1)
        score = over * 1000000.0 + fill
        score[cnt >= P] = np.inf
        t = int(np.argmin(score))
        assign[n] = t
        cnt[t] += 1
        s0[t] += a
        s1[t] += b
    return assign


def _refine(assign, d0, d1, caps, iters=3000):
    """1-1 swap local search: move bucket sums under their caps."""
    ntile = len(caps)
    s0 = np.bincount(assign, weights=d0, minlength=ntile).astype(np.int64)
    s1 = np.bincount(assign, weights=d1, minlength=ntile).astype(np.int64)
    for _ in range(iters):
        viol = np.maximum(s0 - caps, 0) + np.maximum(s1 - caps, 0)
        tb = int(np.argmax(viol))
        if viol[tb] == 0:
            break
        members = np.nonzero(assign == tb)[0]
        slack = np.minimum(caps - s0, caps - s1)
        improved = False
        for tt in np.argsort(-slack)[:4]:
            tt = int(tt)
            if tt == tb:
                continue
            cand = np.nonzero(assign == tt)[0]
            du0 = d0[members][:, None]
            du1 = d1[members][:, None]
            dv0 = d0[cand][None, :]
            dv1 = d1[cand][None, :]
            nb = (np.maximum(s0[tb] - du0 + dv0 - caps[tb], 0)
                  + np.maximum(s1[tb] - du1 + dv1 - caps[tb], 0)
                  + np.maximum(s0[tt] + du0 - dv0 - caps[tt], 0)
                  + np.maximum(s1[tt] + du1 - dv1 - caps[tt], 0))
            i, j = np.unravel_index(np.argmin(nb), nb.shape)
            cur = (viol[tb] + max(s0[tt] - caps[tt], 0)
                   + max(s1[tt] - caps[tt], 0))
            if nb[i, j] < cur:
                u, v = members[i], cand[j]
                assign[u], assign[v] = tt, tb
                s0[tb] += d0[v] - d0[u]
                s1[tb] += d1[v] - d1[u]
                s0[tt] += d0[u] - d0[v]
                s1[tt] += d1[u] - d1[v]
                improved = True
                break
        if not improved:
            break
    return assign


def _preprocess(x, edge_index, n_core=NCORE):
    N = x.shape[0]
    assert N % n_core == 0
    SH = N // n_core
    NT = (SH + P - 1) // P
    NPAD = NT * P
    PAIRS = n_core * NPAD // 2
    PWROWS = PAIRS // NWIN
    assert PWROWS < 32768
    n_tb = (NT + TBS - 1) // TBS

    # self-loops are handled separately on-device (local diagonal term);
    # the gather stream holds only the real edges.
    src = np.asarray(edge_index[0], dtype=np.int64)
    dst = np.asarray(edge_index[1], dtype=np.int64)

    deg = np.bincount(dst, minlength=N).astype(np.int64) + 1
    dinv = np.zeros(N, dtype=np.float32)
    nz = deg > 0
    dinv[nz] = 1.0 / np.sqrt(deg[nz].astype(np.float32))

    # node -> table row. The table is laid out in two half-shard chunks
    # (chunk = AllGather unit = int16 index window): chunk c holds every
    # core's rows [c*NPAD/2, (c+1)*NPAD/2), concatenated core-major.
    # Within each (core, chunk) the node->tile assignment is balanced so
    # every (tile, window) gather bucket packs just under a multiple of
    # 128 across all cores — minimizing gather-slot padding.
    HNP = NPAD // 2
    HT_ = HNP // P
    nodes = np.arange(N, dtype=np.int64)
    k_of = nodes // SH
    i_nat = nodes % SH
    c_of = i_nat // HNP            # chunk (window) membership: natural

    # per-node in-degree split by source window (fixed by c_of of sources)
    wdeg = np.bincount(dst * 2 + c_of[src], minlength=2 * N).reshape(N, 2)

    # bucket cap: largest multiple of 128 below the per-(core,window) mean,
    # plus a few shared overflow tiles (cap+128) sized for the worst core
    cap = (len(src) // n_core // (2 * NT)) // P * P + P
    novf = 0
    for k in range(n_core):
        for c in range(2):
            sel = (k_of[dst] == k) & (c_of[dst] == c)
            for w in range(2):
                tot = int(np.sum(sel & (c_of[src] == w)))
                novf = max(novf, -(-max(0, tot - HT_ * cap) // P))
    caps = np.full(HT_, cap, dtype=np.int64)
    caps[:novf] += P
    i_of = np.empty(N, dtype=np.int64)
    for k in range(n_core):
        for c in range(2):
            sel = np.nonzero((k_of == k) & (c_of == c))[0]
            d0s = wdeg[sel, 0].astype(np.int64)
            d1s = wdeg[sel, 1].astype(np.int64)
            a = _balance(d0s, d1s, HT_, caps)
            a = _refine(a, d0s, d1s, caps)
            order = np.argsort(a, kind="stable")
            pos = np.empty(len(sel), dtype=np.int64)
            pos[order] = np.arange(len(sel))
            # nodes sorted by tile, sequential rel within tile
            tile_sorted = a[order]
            rel = np.arange(len(sel)) - np.searchsorted(
                tile_sorted, tile_sorted, side="left")
            i_local = tile_sorted * P + rel
            i_of[sel[order]] = c * HNP + i_local
    tg = c_of * (n_core * HNP) + k_of * HNP + (i_of - c_of * HNP)

    core_of = dst // SH
    dst_pos = i_of[dst]
    tile_of = dst_pos // P
    dst_rel = dst_pos % P
    src_pair = tg[src] // 2
    src_par = tg[src] % 2
    win_of = src_pair // PWROWS
    src_loc = (src_pair - win_of * PWROWS).astype(np.int64)
    tb_of = tile_of // TBS

    NKEY = n_tb * NWIN * NT
    key_all = ((tb_of * NWIN + win_of) * NT + tile_of).astype(np.int64)

    # common (max over cores) chunk counts per (tb, w, t)
    nch = np.zeros((n_core, NKEY), dtype=np.int64)
    for k in range(n_core):
        sel = np.nonzero(core_of == k)[0]
        cnt = np.bincount(key_all[sel], minlength=NKEY)
        nch[k] = (cnt + P - 1) // P
    common = nch.max(axis=0)                   # chunks per key
    SLOTS = int(common.sum()) * P
    base = np.zeros(NKEY + 1, dtype=np.int64)
    np.cumsum(common * P, out=base[1:])

    IDX = np.zeros((n_core, P, SLOTS // 16), dtype=np.int16)
    DRELE = np.full((n_core, P, SLOTS // P), -1.0, dtype=NPBF16)
    DRELO = np.full((n_core, P, SLOTS // P), -1.0, dtype=NPBF16)
    for k in range(n_core):
        sel = np.nonzero(core_of == k)[0]
        key = key_all[sel]
        # sort by (bucket, source row) -- ascending rows inside each
        # bucket give the HBM far better locality during the gather
        order = np.lexsort((src_loc[sel], key))
        e = sel[order]
        ekey = key[order]
        starts = np.concatenate(
            ([0], np.cumsum(np.bincount(ekey, minlength=NKEY))))[:-1]
        runpos = np.arange(len(e)) - starts[ekey]
        slot = base[ekey] + runpos
        loc = src_loc[e]
        par = src_par[e]
        rel = dst_rel[e].astype(np.float32).astype(NPBF16)
        for rep in range(8):
            IDX[k, 16 * rep + (slot % 16), slot // 16] = loc
        ev = par == 0
        DRELE[k, slot[ev] % P, slot[ev] // P] = rel[ev]
        od = ~ev
        DRELO[k, slot[od] % P, slot[od] // P] = rel[od]

    # compile-time plan per tile-block: gather spans per (tb, w) split to
    # GMAX, matmul cols per dest tile
    colb = base // P                            # key -> starting col
    tb_plans = []
    for tb in range(n_tb):
        gathers = []
        for w in range(NWIN):
            k0 = (tb * NWIN + w) * NT
            n = int(common[k0:k0 + NT].sum()) * P
            if n:
                gathers.append((w, int(colb[k0]), n))
        tiles = []
        for ti in range(TBS):
            t = tb * TBS + ti
            if t >= NT:
                break
            cols = []
            for w in range(NWIN):
                kk = (tb * NWIN + w) * NT + t
                cols.extend(range(int(colb[kk]),
                                  int(colb[kk]) + int(common[kk])))
            tiles.append((t, cols))
        tb_plans.append(dict(gathers=gathers, tiles=tiles))

    # per-core dinv arrays and permuted features
    HALF = (NT + 1) // 2
    dinv_pc = np.zeros((n_core, P, NT), dtype=np.float32)
    dinv_fold = np.zeros((n_core, P, HALF * P), dtype=NPBF16)
    xT = np.zeros((n_core, x.shape[1], NPAD), dtype=NPBF16)
    xf = np.asarray(x, dtype=np.float32)
    for k in range(n_core):
        ns = np.arange(k * SH, (k + 1) * SH)
        dv = np.zeros(NPAD, dtype=np.float32)
        dv[i_of[ns]] = dinv[ns]
        pos = np.arange(NPAD)
        dinv_pc[k, pos % P, pos // P] = dv
        first = dv[:HALF * P].astype(NPBF16)
        second = dv[HALF * P:].astype(NPBF16)
        dinv_fold[k, 0:64, :] = np.broadcast_to(first, (64, HALF * P))
        dinv_fold[k, 64:128, :len(second)] = np.broadcast_to(
            second, (64, len(second)))
        xT[k][:, i_of[ns]] = xf[ns].T.astype(NPBF16)

    iota = np.broadcast_to(np.arange(P, dtype=np.float32),
                           (P, P)).astype(NPBF16).copy()

    return dict(
        N=N, SH=SH, NT=NT, NPAD=NPAD, PWROWS=PWROWS,
        SLOTS=SLOTS, HALF=HALF, n_tb=n_tb, tb_plans=tb_plans,
        IDX=IDX, DRELE=DRELE, DRELO=DRELO,
        dinv_pc=dinv_pc, dinv_fold=dinv_fold, xT=xT, iota=iota,
        IOF=i_of,
    )


# ---------------------------------------------------------------------------
# Device kernel builder
# ---------------------------------------------------------------------------

def build_gcn_module(meta, F, D, C, n_core=NCORE, repeat=1, ablate=None):
    NT = meta["NT"]
    NPAD = meta["NPAD"]
    PWROWS = meta["PWROWS"]
    SLOTS = meta["SLOTS"]
    HALF = meta["HALF"]
    n_tb = meta["n_tb"]
    tb_plans = meta["tb_plans"]
    TROWS = n_core * NPAD
    KF = F // P
    assert D == 64 and C <= 64

    nc = bacc.Bacc(num_devices=n_core, dynamic_dma_scratch_size=SCRATCH,
                   num_swdge_queues=NQ)

    xT = nc.declare_dram_parameter("xT", [F, NPAD], BF16, isOutput=False)
    idx = nc.declare_dram_parameter("idx", [P, SLOTS // 16], I16,
                                    isOutput=False)
    drele = nc.declare_dram_parameter("drele", [P, SLOTS // P], BF16,
                                      isOutput=False)
    drelo = nc.declare_dram_parameter("drelo", [P, SLOTS // P], BF16,
                                      isOutput=False)
    dinv = nc.declare_dram_parameter("dinv", [P, NT], F32, isOutput=False)
    dinvf = nc.declare_dram_parameter("dinvf", [P, HALF * P], BF16,
                                      isOutput=False)
    w1 = nc.declare_dram_parameter("W1", [F, D], BF16, isOutput=False)
    b1f = nc.declare_dram_parameter("b1f", [P, 1], F32, isOutput=False)
    w2 = nc.declare_dram_parameter("W2p", [P, D], BF16, isOutput=False)
    b2r = nc.declare_dram_parameter("b2r", [P, C], F32, isOutput=False)
    iot = nc.declare_dram_parameter("iota", [P, P], BF16, isOutput=False)
    idn = nc.declare_dram_parameter("ident", [P, P], BF16, isOutput=False)
    out = nc.declare_dram_parameter("out", [P, NT, C], F32, isOutput=True)

    t1_shard = nc.dram_tensor("t1_shard", [NPAD, D], BF16)
    t1_full = nc.dram_tensor("t1_full", [TROWS, D], BF16,
                             addr_space="Shared")
    t2_shard = nc.dram_tensor("t2_shard", [NPAD, D], BF16)
    t2_full = nc.dram_tensor("t2_full", [TROWS, D], BF16,
                             addr_space="Shared")

    rg = [list(range(n_core))]
    Copy = mybir.ActivationFunctionType.Copy

    def fold_slice(t):
        return (0, t * P) if t < HALF else (64, (t - HALF) * P)

    HNP = NPAD // 2

    def all_gather(shard, full, c):
        """AllGather chunk c (half-shard): issued as soon as that half of
        the shard table is written, so the collective overlaps compute."""
        sl = shard[c * HNP:(c + 1) * HNP, :]
        if ablate == "nocoll":
            nc.sync.dma_start(
                out=full[c * n_core * HNP:c * n_core * HNP + HNP, :], in_=sl)
        else:
            nc.gpsimd.collective_compute(
                "AllGather", mybir.AluOpType.bypass, replica_groups=rg,
                ins=[sl],
                outs=[full[c * n_core * HNP:(c + 1) * n_core * HNP, :]])

    with tile.TileContext(nc) as tc:
        with (
            tc.tile_pool(name="const", bufs=1) as cpool,
            tc.tile_pool(name="stream", bufs=1) as ipool,
            tc.tile_pool(name="stripm", bufs=4) as mpool,
            tc.tile_pool(name="strips", bufs=2) as spool,
            tc.tile_pool(name="work", bufs=3) as wpool,
            tc.tile_pool(name="big", bufs=1) as bpool,
            tc.tile_pool(name="psA", bufs=2, space="PSUM") as psA,
            tc.tile_pool(name="psB", bufs=2, space="PSUM") as psB,
        ):
            nc.gpsimd.load_library(library_config.mlp)

            # ---- constants ----
            w1t = cpool.tile([P, KF, D], BF16, tag="w1t")
            nc.sync.dma_start(
                out=w1t[:], in_=w1.rearrange("(k p) d -> p k d", p=P))
            w2t = cpool.tile([P, D], BF16, tag="w2t")
            nc.sync.dma_start(out=w2t[:], in_=w2[:, :])
            b1t = cpool.tile([P, 1], F32, tag="b1t")
            nc.sync.dma_start(out=b1t[:], in_=b1f[:, :])
            b2t = cpool.tile([P, C], F32, tag="b2t")
            nc.sync.dma_start(out=b2t[:], in_=b2r[:, :])
            dinvt = cpool.tile([P, NT], F32, tag="dinvt")
            nc.sync.dma_start(out=dinvt[:], in_=dinv[:, :])
            dinvft = cpool.tile([P, HALF * P], BF16, tag="dinvft")
            nc.sync.dma_start(out=dinvft[:], in_=dinvf[:, :])
            iota = cpool.tile([P, P], BF16, tag="iota")
            nc.sync.dma_start(out=iota[:], in_=iot[:, :])
            ident = cpool.tile([P, P], BF16, tag="ident")
            nc.sync.dma_start(out=ident[:], in_=idn[:, :])
            idxt = ipool.tile([P, SLOTS // 16], I16, tag="idxt")
            nc.sync.dma_start(out=idxt[:], in_=idx[:, :])
            drelet = ipool.tile([P, SLOTS // P], BF16, tag="drelet")
            nc.sync.dma_start(out=drelet[:], in_=drele[:, :])
            drelot = ipool.tile([P, SLOTS // P], BF16, tag="drelot")
            nc.sync.dma_start(out=drelot[:], in_=drelo[:, :])

            qctr = [0]
            # tile-blocks of window-0 gathers in flight ahead of
            # window-1+processing; needs KPIPE+2 msg pool buffers
            KPIPE = int(os.environ.get("GCN_KPIPE", "2"))

            def gather_stream(plan, msg, table, col_base, win):
                if ablate in ("nogather", "noedge"):
                    return
                pairs = table.rearrange("(a two) d -> a (two d)", two=2)
                for (w, col0, n) in plan["gathers"]:
                    if w != win:
                        continue
                    for s in range(0, n, GMAX):
                        m = min(GMAX, n - s)
                        c = col0 - col_base + s // P
                        nc.gpsimd.dma_gather(
                            out_ap=msg[:, c:c + m // P, :],
                            in_ap=pairs[w * PWROWS:(w + 1) * PWROWS, :],
                            idxs_ap=idxt[:, (col0 * P + s) // 16:
                                         (col0 * P + s + m) // 16],
                            num_idxs=m, num_idxs_reg=m, elem_size=ROWW,
                            queue_num=qctr[0] % NQ)
                        qctr[0] += 1

            def layer_loop(table, process_tb):
                """Window-pipelined tb loop: window-0 gathers run KPIPE
                tile-blocks ahead so a late window-1 AllGather chunk never
                head-of-line-blocks the gpsimd queue."""
                state = {}
                for i in range(n_tb + KPIPE):
                    if i < n_tb:
                        plan = tb_plans[i]
                        col_base = min(c0 for _, c0, _ in plan["gathers"])
                        cols_tb = sum(
                            n for _, _, n in plan["gathers"]) // P
                        msg = mpool.tile([P, cols_tb, ROWW], BF16,
                                         tag="msg")
                        gather_stream(plan, msg, table, col_base, 0)
                        state[i] = (plan, col_base, cols_tb, msg)
                    j = i - KPIPE
                    if 0 <= j < n_tb:
                        plan, col_base, cols_tb, msg = state.pop(j)
                        gather_stream(plan, msg, table, col_base, 1)
                        process_tb(j, plan, col_base, cols_tb, msg)

            def build_st(drelt, plan, col_base, cols_tb, tag):
                st = spool.tile([P, cols_tb, P], BF16, tag=tag)
                nc.vector.tensor_tensor(
                    out=st[:],
                    in0=drelt[:, col_base:col_base + cols_tb, None
                              ].broadcast_to([P, cols_tb, P]),
                    in1=iota[:, None, :].broadcast_to([P, cols_tb, P]),
                    op=mybir.AluOpType.is_equal)
                return st

            HT = HALF  # tiles per half (49)

            def compute_body():
                # ---- phase A: t1_shard = dinv * (x @ W1), compact bf16 ----
                # AllGather each half-shard as soon as it is written so the
                # first collective overlaps the second half's compute.
                with tc.tile_pool(name="xw", bufs=3) as xpool:
                    for c in range(2):
                        for i in range(c * HT, min((c + 1) * HT, NT)):
                            xt = xpool.tile([P, KF, P], BF16, tag="xt")
                            nc.sync.dma_start(
                                out=xt[:],
                                in_=xT.rearrange("(k p) n -> p k n", p=P)[
                                    :, :, i * P:(i + 1) * P])
                            ph = psA.tile([P, D], F32, tag="ph")
                            for k in range(KF):
                                nc.tensor.matmul(
                                    out=ph[:], lhsT=xt[:, k, :],
                                    rhs=w1t[:, k, :],
                                    start=(k == 0), stop=(k == KF - 1))
                            hs = xpool.tile([P, D], BF16, tag="hs")
                            nc.scalar.activation(
                                out=hs[:], in_=ph[:], func=Copy,
                                scale=dinvt[:, i:i + 1])
                            nc.sync.dma_start(
                                out=t1_shard[i * P:(i + 1) * P, :], in_=hs[:])
                        all_gather(t1_shard, t1_full, c)

                # ==================== LAYER 1 ====================
                # t2/AllGather2 for the first half is emitted as soon as the
                # tile-blocks covering tiles 0..HT-1 are aggregated, hiding
                # the collective under the second half's gather stream.
                h1buf = bpool.tile([P, HALF * P], BF16, tag="bigH")
                t2buf = bpool.tile([P, NT, D], BF16, tag="t2buf")

                def finalize_half(c):
                    fp = 64 * c
                    nc.vector.tensor_tensor(
                        out=h1buf[fp:fp + 64, :], in0=h1buf[fp:fp + 64, :],
                        in1=dinvft[fp:fp + 64, :], op=mybir.AluOpType.mult)
                    nc.scalar.activation(
                        out=h1buf[fp:fp + 64, :], in_=h1buf[fp:fp + 64, :],
                        func=mybir.ActivationFunctionType.Relu,
                        bias=b1t[fp:fp + 64, 0:1])
                    for t in range(c * HT, min((c + 1) * HT, NT)):
                        _, fc = fold_slice(t)
                        po = psB.tile([P, D], F32, tag="po")
                        nc.tensor.matmul(
                            out=po[:], lhsT=h1buf[fp:fp + 64, fc:fc + P],
                            rhs=w2t[fp:fp + 64, :], start=True, stop=True)
                        nc.scalar.activation(
                            out=t2buf[:, t, :], in_=po[:], func=Copy,
                            scale=dinvt[:, t:t + 1])
                    lo = c * HT
                    hi = min((c + 1) * HT, NT)
                    nc.sync.dma_start(
                        out=t2_shard.rearrange("(q p) d -> p q d", p=P)[
                            :, lo:hi, :],
                        in_=t2buf[:, lo:hi, :])
                    all_gather(t2_shard, t2_full, c)

                tb_half = (HT + TBS - 1) // TBS - 1   # last tb covering half0

                def process_l1(tb, plan, col_base, cols_tb, msg):
                    ste = build_st(drelet, plan, col_base, cols_tb, "ste")
                    sto = build_st(drelot, plan, col_base, cols_tb, "sto")
                    ntt = len(plan["tiles"])
                    t0_ = plan["tiles"][0][0]
                    selfb = wpool.tile([P, TBS, D], BF16, tag="selfb")
                    nc.sync.dma_start(
                        out=selfb[:, 0:ntt, :],
                        in_=t1_shard[t0_ * P:(t0_ + ntt) * P, :].rearrange(
                            "(q p) d -> p q d", p=P))
                    for (t, cols) in plan["tiles"]:
                        if ablate in ("noedge", "nomm"):
                            cols = []
                        pt = psB.tile([D, P], F32, tag="pt")
                        for ci, col in enumerate(cols):
                            cc = col - col_base
                            nc.tensor.matmul(
                                out=pt[:], lhsT=msg[:, cc, 0:D],
                                rhs=ste[:, cc, :],
                                start=(ci == 0), stop=False)
                            nc.tensor.matmul(
                                out=pt[:], lhsT=msg[:, cc, D:ROWW],
                                rhs=sto[:, cc, :],
                                start=False, stop=False)
                        # self-loop diagonal term from the local shard
                        nc.tensor.matmul(
                            out=pt[:], lhsT=selfb[:, t - t0_, :],
                            rhs=ident[:],
                            start=(len(cols) == 0), stop=True)
                        fp, fc = fold_slice(t)
                        nc.scalar.activation(
                            out=h1buf[fp:fp + 64, fc:fc + P], in_=pt[:],
                            func=Copy)
                    if tb == tb_half:
                        finalize_half(0)

                layer_loop(t1_full, process_l1)
                finalize_half(1)

                # ==================== LAYER 2 ====================
                obuf = bpool.tile([P, NT, C], F32, tag="bigA")
                mt = cpool.tile([P, NT], F32, tag="mt")
                sums = cpool.tile([P, NT], F32, tag="sums")
                escr = cpool.tile([P, C], F32, tag="escr")
                lst = cpool.tile([P, NT], F32, tag="lst")

                def softmax_half(c):
                    # out[:, lo:hi] = log_softmax(obuf[:, lo:hi] + b2)
                    lo = c * HT
                    hi = min((c + 1) * HT, NT)
                    ob = obuf[:, lo:hi, :]
                    nt = hi - lo
                    nc.vector.tensor_tensor(
                        out=ob, in0=ob,
                        in1=b2t[:, None, :].broadcast_to([P, nt, C]),
                        op=mybir.AluOpType.add)
                    nc.vector.tensor_reduce(
                        out=mt[:, lo:hi], in_=ob, axis=mybir.AxisListType.X,
                        op=mybir.AluOpType.max)
                    nc.vector.tensor_tensor(
                        out=ob, in0=ob,
                        in1=mt[:, lo:hi, None].broadcast_to([P, nt, C]),
                        op=mybir.AluOpType.subtract)
                    eb = cpool.tile([P, HT, C], F32, tag="ebig")
                    nc.scalar.activation(
                        out=eb[:, 0:nt, :], in_=ob,
                        func=mybir.ActivationFunctionType.Exp)
                    nc.vector.tensor_reduce(
                        out=sums[:, lo:hi], in_=eb[:, 0:nt, :],
                        axis=mybir.AxisListType.X, op=mybir.AluOpType.add)
                    nc.scalar.activation(
                        out=lst[:, lo:hi], in_=sums[:, lo:hi],
                        func=mybir.ActivationFunctionType.Ln)
                    nc.vector.tensor_tensor(
                        out=ob, in0=ob,
                        in1=lst[:, lo:hi, None].broadcast_to([P, nt, C]),
                        op=mybir.AluOpType.subtract)
                    nc.sync.dma_start(out=out[:, lo:hi, :],
                                      in_=obuf[:, lo:hi, :])

                def process_l2(tb, plan, col_base, cols_tb, msg):
                    ste = build_st(drelet, plan, col_base, cols_tb, "ste")
                    sto = build_st(drelot, plan, col_base, cols_tb, "sto")
                    ntt = len(plan["tiles"])
                    t0_ = plan["tiles"][0][0]
                    selfb = wpool.tile([P, TBS, D], BF16, tag="selfb2")
                    nc.sync.dma_start(
                        out=selfb[:, 0:ntt, :],
                        in_=t2_shard[t0_ * P:(t0_ + ntt) * P, :].rearrange(
                            "(q p) d -> p q d", p=P))
                    for (t, cols) in plan["tiles"]:
                        if ablate in ("noedge", "nomm"):
                            cols = []
                        pa = psB.tile([P, C], F32, tag="pa")
                        for ci, col in enumerate(cols):
                            cc = col - col_base
                            nc.tensor.matmul(
                                out=pa[:], lhsT=ste[:, cc, :],
                                rhs=msg[:, cc, 0:C],
                                start=(ci == 0), stop=False)
                            nc.tensor.matmul(
                                out=pa[:], lhsT=sto[:, cc, :],
                                rhs=msg[:, cc, D:D + C],
                                start=False, stop=False)
                        nc.tensor.matmul(
                            out=pa[:], lhsT=ident[:],
                            rhs=selfb[:, t - t0_, 0:C],
                            start=(len(cols) == 0), stop=True)
                        nc.scalar.activation(
                            out=obuf[:, t, :], in_=pa[:], func=Copy,
                            scale=dinvt[:, t:t + 1])
                    if tb == tb_half:
                        softmax_half(0)

                layer_loop(t2_full, process_l2)
                softmax_half(1)

            for _rep in range(repeat):
                compute_body()

    return nc


# ---------------------------------------------------------------------------
# Entry point
# ---------------------------------------------------------------------------

def prepare(x, edge_index, W1, b1, W2, b2, repeat=1, ablate=None):
    """Build (nc, in_maps, meta) without running — shared by kernel() and
    external benchmarking harnesses."""
    x = np.asarray(x, dtype=np.float32)
    W1 = np.asarray(W1, dtype=np.float32)
    b1 = np.asarray(b1, dtype=np.float32)
    W2 = np.asarray(W2, dtype=np.float32)
    b2 = np.asarray(b2, dtype=np.float32)

    F, D = W1.shape
    C = W2.shape[1]

    meta = _preprocess(x, edge_index)

    nc = build_gcn_module(meta, F, D, C, repeat=repeat, ablate=ablate)
    nc.finalize()

    W2p = np.zeros((P, D), dtype=NPBF16)
    W2p[0:64, :C] = W2.astype(NPBF16)
    W2p[64:128, :C] = W2.astype(NPBF16)
    b1fold = np.empty((P, 1), dtype=np.float32)
    b1fold[0:64, 0] = b1
    b1fold[64:128, 0] = b1
    b2r = np.broadcast_to(b2, (P, C)).astype(np.float32).copy()
    in_maps = []
    for k in range(NCORE):
        in_maps.append({
            "xT": meta["xT"][k],
            "idx": meta["IDX"][k],
            "drele": meta["DRELE"][k],
            "drelo": meta["DRELO"][k],
            "dinv": meta["dinv_pc"][k],
            "dinvf": meta["dinv_fold"][k],
            "W1": W1.astype(NPBF16), "b1f": b1fold, "W2p": W2p, "b2r": b2r,
            "iota": meta["iota"],
            "ident": np.eye(P, dtype=np.float32).astype(NPBF16),
        })
    return nc, in_maps, meta


def kernel(x, edge_index, W1, b1, W2, b2):
    N = np.asarray(x).shape[0]
    C = np.asarray(W2).shape[1]

    nc, in_maps, meta = prepare(x, edge_index, W1, b1, W2, b2)
    NT, SH = meta["NT"], meta["SH"]

    res = run_bass_kernel_spmd(
        nc, in_maps, core_ids=list(range(NCORE)),
        trace=os.environ.get("GCN_TRACE") == "1")
    kernel.last_results = res

    out = np.empty((N, C), dtype=np.float32)
    iof = meta["IOF"]
    for k in range(NCORE):
        ok = np.asarray(res.results[k]["out"]).reshape(P, NT, C)
        pos = iof[k * SH:(k + 1) * SH]
        out[k * SH:(k + 1) * SH] = ok[pos % P, pos // P, :]
    return out


# revision 61
# speedup vs baseline: 1.0803x; 1.0803x over previous
"""Trainium2 Bass kernel for a 2-layer Kipf GCN (nn_KipfGCN_1743756722177).

Strategy (8 NeuronCores, SPMD):
  - Nodes sharded contiguously across cores (12500 each, padded to 12544).
    Edges (incl. self-loops) partitioned by destination core.
  - Symmetric normalization dinv[src]*dinv[dst] is folded into table row
    scaling (src side) and per-partition output scaling (dst side); no
    per-edge multiplies.
  - Per layer, a compact bf16 node-feature table ([N,64] rows) is computed
    shard-wise and AllGathered so every core can gather any row.
  - Messages are fetched with dma_gather (int16 indices) at PAIR
    granularity: one 256-byte gather row holds two consecutive nodes'
    features (64+64 bf16). The pair parity is resolved during aggregation
    by two parity-masked one-hot matrices, so no bytes are wasted and the
    AllGather moves half the data of an fp32 layout.
  - Edge slots are grouped by (tile-block, window, dest-tile), sorted by
    source row within each bucket (HBM locality), padded to 128-slot
    chunks (pad slots gather row 0 and are masked out of the aggregation).
    The node->tile assignment is balanced host-side (2D bin packing of
    per-window in-degrees) so every bucket packs just under a multiple of
    128 on all 8 cores: slot padding drops from 25% to 1.4%. This matters
    because dma_gather descriptor generation runs serially on the gpsimd
    engine at ~4ns/slot — the kernel's bottleneck. Gather calls
    round-robin over 4 SWDGE queues (per-queue rings / Q7 core pairs).
  - The per-layer AllGather is split in two half-shard collectives issued
    as soon as each half of the table is written, overlapping them with
    phase-A compute (layer 1) and the previous layer's gather stream
    (layer 2); the halves coincide with the two int16 index windows.
  - Aggregation per 128-node destination tile: for each 128-edge chunk,
    bf16 one-hot matrices S_e/S_o (S[e,j] = dst_rel[e]==j, masked to
    even/odd-parity slots, built by the vector engine from an iota
    constant) are contracted on the tensor engine with fp32 PSUM
    accumulation against the corresponding half of the gathered pair row.
  - All floating-point math runs on device; the host only does integer
    index prep, sharding/layout, and de-padding of the output.
"""

import os

import numpy as np

import concourse.bass as bass
import concourse.bacc as bacc
import concourse.mybir as mybir
import concourse.tile as tile
from concourse import library_config
from concourse.bass_utils import run_bass_kernel_spmd

P = 128
NCORE = 8
TBS = 3                       # dest tiles per tile-block
NWIN = 2                      # int16 index windows over the pair table
GMAX = 1024                   # max indices per dma_gather call (HW limit)
SCRATCH = 32768               # SWDGE ring bytes/partition (2048 descs)
NQ = 4                        # SWDGE queues; gather calls round-robin
F32 = mybir.dt.float32
BF16 = mybir.dt.bfloat16
I16 = mybir.dt.int16
NPBF16 = mybir.dt.np(mybir.dt.bfloat16)
ROWW = 128                    # gather row width: 2 nodes x 64 feats bf16


# ---------------------------------------------------------------------------
# Host-side preprocessing
# ---------------------------------------------------------------------------

def _balance(d0, d1, ntile, caps):
    """Greedy 2D bin packing: assign nodes (with per-window in-degrees
    d0/d1) to ntile tiles of 128 nodes so both per-window bucket sums
    stay at/under caps[t] (multiples of 128). Minimizes gather padding."""
    nn = len(d0)
    order = np.argsort(-(d0 + d1), kind="stable")
    cnt = np.zeros(ntile, dtype=np.int64)
    s0 = np.zeros(ntile, dtype=np.int64)
    s1 = np.zeros(ntile, dtype=np.int64)
    assign = np.empty(nn, dtype=np.int64)
    # proportional fill targets: bins fill toward their cap share, so
    # larger (overflow) bins genuinely absorb the over-mean excess
    t0 = caps * (max(int(d0.sum()), 1) / caps.sum())
    t1 = caps * (max(int(d1.sum()), 1) / caps.sum())
    for n in order:
        a, b = d0[n], d1[n]
        over = (np.maximum(s0 + a - caps, 0) + np.maximum(s1 + b - caps, 0))
        fill = np.maximum((s0 + a) / t0, (s1 + b) / t1)
        score = over * 1000000.0 + fill
        score[cnt >= P] = np.inf
        t = int(np.argmin(score))
        assign[n] = t
        cnt[t] += 1
        s0[t] += a
        s1[t] += b
    return assign


def _refine(assign, d0, d1, caps, iters=3000):
    """1-1 swap local search: move bucket sums under their caps."""
    ntile = len(caps)
    s0 = np.bincount(assign, weights=d0, minlength=ntile).astype(np.int64)
    s1 = np.bincount(assign, weights=d1, minlength=ntile).astype(np.int64)
    for _ in range(iters):
        viol = np.maximum(s0 - caps, 0) + np.maximum(s1 - caps, 0)
        tb = int(np.argmax(viol))
        if viol[tb] == 0:
            break
        members = np.nonzero(assign == tb)[0]
        slack = np.minimum(caps - s0, caps - s1)
        improved = False
        for tt in np.argsort(-slack)[:4]:
            tt = int(tt)
            if tt == tb:
                continue
            cand = np.nonzero(assign == tt)[0]
            du0 = d0[members][:, None]
            du1 = d1[members][:, None]
            dv0 = d0[cand][None, :]
            dv1 = d1[cand][None, :]
            nb = (np.maximum(s0[tb] - du0 + dv0 - caps[tb], 0)
                  + np.maximum(s1[tb] - du1 + dv1 - caps[tb], 0)
                  + np.maximum(s0[tt] + du0 - dv0 - caps[tt], 0)
                  + np.maximum(s1[tt] + du1 - dv1 - caps[tt], 0))
            i, j = np.unravel_index(np.argmin(nb), nb.shape)
            cur = (viol[tb] + max(s0[tt] - caps[tt], 0)
                   + max(s1[tt] - caps[tt], 0))
            if nb[i, j] < cur:
                u, v = members[i], cand[j]
                assign[u], assign[v] = tt, tb
                s0[tb] += d0[v] - d0[u]
                s1[tb] += d1[v] - d1[u]
                s0[tt] += d0[u] - d0[v]
                s1[tt] += d1[u] - d1[v]
                improved = True
                break
        if not improved:
            break
    return assign


def _preprocess(x, edge_index, n_core=NCORE):
    N = x.shape[0]
    assert N % n_core == 0
    SH = N // n_core
    NT = (SH + P - 1) // P
    NPAD = NT * P
    PAIRS = n_core * NPAD // 2
    PWROWS = PAIRS // NWIN
    assert PWROWS < 32768
    n_tb = (NT + TBS - 1) // TBS

    # self-loops are handled separately on-device (local diagonal term);
    # the gather stream holds only the real edges.
    src = np.asarray(edge_index[0], dtype=np.int64)
    dst = np.asarray(edge_index[1], dtype=np.int64)

    deg = np.bincount(dst, minlength=N).astype(np.int64) + 1
    dinv = np.zeros(N, dtype=np.float32)
    nz = deg > 0
    dinv[nz] = 1.0 / np.sqrt(deg[nz].astype(np.float32))

    # node -> table row. The table is laid out in two half-shard chunks
    # (chunk = AllGather unit = int16 index window): chunk c holds every
    # core's rows [c*NPAD/2, (c+1)*NPAD/2), concatenated core-major.
    # Within each (core, chunk) the node->tile assignment is balanced so
    # every (tile, window) gather bucket packs just under a multiple of
    # 128 across all cores — minimizing gather-slot padding.
    HNP = NPAD // 2
    HT_ = HNP // P
    nodes = np.arange(N, dtype=np.int64)
    k_of = nodes // SH
    i_nat = nodes % SH
    c_of = i_nat // HNP            # chunk (window) membership: natural

    # per-node in-degree split by source window (fixed by c_of of sources)
    wdeg = np.bincount(dst * 2 + c_of[src], minlength=2 * N).reshape(N, 2)

    # bucket cap: largest multiple of 128 below the per-(core,window) mean,
    # plus a few shared overflow tiles (cap+128) sized for the worst core
    cap = (len(src) // n_core // (2 * NT)) // P * P + P
    novf = 0
    for k in range(n_core):
        for c in range(2):
            sel = (k_of[dst] == k) & (c_of[dst] == c)
            for w in range(2):
                tot = int(np.sum(sel & (c_of[src] == w)))
                novf = max(novf, -(-max(0, tot - HT_ * cap) // P))
    caps = np.full(HT_, cap, dtype=np.int64)
    caps[:novf] += P
    i_of = np.empty(N, dtype=np.int64)
    for k in range(n_core):
        for c in range(2):
            sel = np.nonzero((k_of == k) & (c_of == c))[0]
            d0s = wdeg[sel, 0].astype(np.int64)
            d1s = wdeg[sel, 1].astype(np.int64)
            a = _balance(d0s, d1s, HT_, caps)
            a = _refine(a, d0s, d1s, caps)
            order = np.argsort(a, kind="stable")
            pos = np.empty(len(sel), dtype=np.int64)
            pos[order] = np.arange(len(sel))
            # nodes sorted by tile, sequential rel within tile
            tile_sorted = a[order]
            rel = np.arange(len(sel)) - np.searchsorted(
                tile_sorted, tile_sorted, side="left")
            i_local = tile_sorted * P + rel
            i_of[sel[order]] = c * HNP + i_local
    tg = c_of * (n_core * HNP) + k_of * HNP + (i_of - c_of * HNP)

    core_of = dst // SH
    dst_pos = i_of[dst]
    tile_of = dst_pos // P
    dst_rel = dst_pos % P
    src_pair = tg[src] // 2
    src_par = tg[src] % 2
    win_of = src_pair // PWROWS
    src_loc = (src_pair - win_of * PWROWS).astype(np.int64)
    tb_of = tile_of // TBS

    NKEY = n_tb * NWIN * NT
    key_all = ((tb_of * NWIN + win_of) * NT + tile_of).astype(np.int64)

    # common (max over cores) chunk counts per (tb, w, t)
    nch = np.zeros((n_core, NKEY), dtype=np.int64)
    for k in range(n_core):
        sel = np.nonzero(core_of == k)[0]
        cnt = np.bincount(key_all[sel], minlength=NKEY)
        nch[k] = (cnt + P - 1) // P
    common = nch.max(axis=0)                   # chunks per key
    SLOTS = int(common.sum()) * P
    base = np.zeros(NKEY + 1, dtype=np.int64)
    np.cumsum(common * P, out=base[1:])

    IDX = np.zeros((n_core, P, SLOTS // 16), dtype=np.int16)
    DRELE = np.full((n_core, P, SLOTS // P), -1.0, dtype=NPBF16)
    DRELO = np.full((n_core, P, SLOTS // P), -1.0, dtype=NPBF16)
    for k in range(n_core):
        sel = np.nonzero(core_of == k)[0]
        key = key_all[sel]
        # sort by (bucket, source row) -- ascending rows inside each
        # bucket give the HBM far better locality during the gather
        order = np.lexsort((src_loc[sel], key))
        e = sel[order]
        ekey = key[order]
        starts = np.concatenate(
            ([0], np.cumsum(np.bincount(ekey, minlength=NKEY))))[:-1]
        runpos = np.arange(len(e)) - starts[ekey]
        slot = base[ekey] + runpos
        loc = src_loc[e]
        par = src_par[e]
        rel = dst_rel[e].astype(np.float32).astype(NPBF16)
        for rep in range(8):
            IDX[k, 16 * rep + (slot % 16), slot // 16] = loc
        ev = par == 0
        DRELE[k, slot[ev] % P, slot[ev] // P] = rel[ev]
        od = ~ev
        DRELO[k, slot[od] % P, slot[od] // P] = rel[od]

    # compile-time plan per tile-block: gather spans per (tb, w) split to
    # GMAX, matmul cols per dest tile
    colb = base // P                            # key -> starting col
    tb_plans = []
    for tb in range(n_tb):
        gathers = []
        for w in range(NWIN):
            k0 = (tb * NWIN + w) * NT
            n = int(common[k0:k0 + NT].sum()) * P
            if n:
                gathers.append((w, int(colb[k0]), n))
        tiles = []
        for ti in range(TBS):
            t = tb * TBS + ti
            if t >= NT:
                break
            cols = []
            for w in range(NWIN):
                kk = (tb * NWIN + w) * NT + t
                cols.extend(range(int(colb[kk]),
                                  int(colb[kk]) + int(common[kk])))
            tiles.append((t, cols))
        tb_plans.append(dict(gathers=gathers, tiles=tiles))

    # per-core dinv arrays and permuted features
    HALF = (NT + 1) // 2
    dinv_pc = np.zeros((n_core, P, NT), dtype=np.float32)
    dinv_fold = np.zeros((n_core, P, HALF * P), dtype=NPBF16)
    xT = np.zeros((n_core, x.shape[1], NPAD), dtype=NPBF16)
    xf = np.asarray(x, dtype=np.float32)
    for k in range(n_core):
        ns = np.arange(k * SH, (k + 1) * SH)
        dv = np.zeros(NPAD, dtype=np.float32)
        dv[i_of[ns]] = dinv[ns]
        pos = np.arange(NPAD)
        dinv_pc[k, pos % P, pos // P] = dv
        first = dv[:HALF * P].astype(NPBF16)
        second = dv[HALF * P:].astype(NPBF16)
        dinv_fold[k, 0:64, :] = np.broadcast_to(first, (64, HALF * P))
        dinv_fold[k, 64:128, :len(second)] = np.broadcast_to(
            second, (64, len(second)))
        xT[k][:, i_of[ns]] = xf[ns].T.astype(NPBF16)

    iota = np.broadcast_to(np.arange(P, dtype=np.float32),
                           (P, P)).astype(NPBF16).copy()

    return dict(
        N=N, SH=SH, NT=NT, NPAD=NPAD, PWROWS=PWROWS,
        SLOTS=SLOTS, HALF=HALF, n_tb=n_tb, tb_plans=tb_plans,
        IDX=IDX, DRELE=DRELE, DRELO=DRELO,
        dinv_pc=dinv_pc, dinv_fold=dinv_fold, xT=xT, iota=iota,
        IOF=i_of,
    )


# ---------------------------------------------------------------------------
# Device kernel builder
# ---------------------------------------------------------------------------

def build_gcn_module(meta, F, D, C, n_core=NCORE, repeat=1, ablate=None):
    NT = meta["NT"]
    NPAD = meta["NPAD"]
    PWROWS = meta["PWROWS"]
    SLOTS = meta["SLOTS"]
    HALF = meta["HALF"]
    n_tb = meta["n_tb"]
    tb_plans = meta["tb_plans"]
    TROWS = n_core * NPAD
    KF = F // P
    assert D == 64 and C <= 64

    nc = bacc.Bacc(num_devices=n_core, dynamic_dma_scratch_size=SCRATCH,
                   num_swdge_queues=NQ)

    xT = nc.declare_dram_parameter("xT", [F, NPAD], BF16, isOutput=False)
    idx = nc.declare_dram_parameter("idx", [P, SLOTS // 16], I16,
                                    isOutput=False)
    drele = nc.declare_dram_parameter("drele", [P, SLOTS // P], BF16,
                                      isOutput=False)
    drelo = nc.declare_dram_parameter("drelo", [P, SLOTS // P], BF16,
                                      isOutput=False)
    dinv = nc.declare_dram_parameter("dinv", [P, NT], F32, isOutput=False)
    dinvf = nc.declare_dram_parameter("dinvf", [P, HALF * P], BF16,
                                      isOutput=False)
    w1 = nc.declare_dram_parameter("W1", [F, D], BF16, isOutput=False)
    b1f = nc.declare_dram_parameter("b1f", [P, 1], F32, isOutput=False)
    w2 = nc.declare_dram_parameter("W2p", [P, D], BF16, isOutput=False)
    b2r = nc.declare_dram_parameter("b2r", [P, C], F32, isOutput=False)
    iot = nc.declare_dram_parameter("iota", [P, P], BF16, isOutput=False)
    idn = nc.declare_dram_parameter("ident", [P, P], BF16, isOutput=False)
    out = nc.declare_dram_parameter("out", [P, NT, C], F32, isOutput=True)

    t1_shard = nc.dram_tensor("t1_shard", [NPAD, D], BF16)
    t1_full = nc.dram_tensor("t1_full", [TROWS, D], BF16,
                             addr_space="Shared")
    t2_shard = nc.dram_tensor("t2_shard", [NPAD, D], BF16)
    t2_full = nc.dram_tensor("t2_full", [TROWS, D], BF16,
                             addr_space="Shared")

    rg = [list(range(n_core))]
    Copy = mybir.ActivationFunctionType.Copy

    def fold_slice(t):
        return (0, t * P) if t < HALF else (64, (t - HALF) * P)

    HNP = NPAD // 2

    def all_gather(shard, full, c):
        """AllGather chunk c (half-shard): issued as soon as that half of
        the shard table is written, so the collective overlaps compute."""
        sl = shard[c * HNP:(c + 1) * HNP, :]
        if ablate == "nocoll":
            nc.sync.dma_start(
                out=full[c * n_core * HNP:c * n_core * HNP + HNP, :], in_=sl)
        else:
            nc.gpsimd.collective_compute(
                "AllGather", mybir.AluOpType.bypass, replica_groups=rg,
                ins=[sl],
                outs=[full[c * n_core * HNP:(c + 1) * n_core * HNP, :]])

    with tile.TileContext(nc) as tc:
        with (
            tc.tile_pool(name="const", bufs=1) as cpool,
            tc.tile_pool(name="stream", bufs=1) as ipool,
            tc.tile_pool(name="stripm", bufs=4) as mpool,
            tc.tile_pool(name="strips", bufs=2) as spool,
            tc.tile_pool(name="work", bufs=3) as wpool,
            tc.tile_pool(name="big", bufs=1) as bpool,
            tc.tile_pool(name="psA", bufs=2, space="PSUM") as psA,
            tc.tile_pool(name="psB", bufs=2, space="PSUM") as psB,
        ):
            nc.gpsimd.load_library(library_config.mlp)

            # ---- constants ----
            w1t = cpool.tile([P, KF, D], BF16, tag="w1t")
            nc.sync.dma_start(
                out=w1t[:], in_=w1.rearrange("(k p) d -> p k d", p=P))
            w2t = cpool.tile([P, D], BF16, tag="w2t")
            nc.sync.dma_start(out=w2t[:], in_=w2[:, :])
            b1t = cpool.tile([P, 1], F32, tag="b1t")
            nc.sync.dma_start(out=b1t[:], in_=b1f[:, :])
            b2t = cpool.tile([P, C], F32, tag="b2t")
            nc.sync.dma_start(out=b2t[:], in_=b2r[:, :])
            dinvt = cpool.tile([P, NT], F32, tag="dinvt")
            nc.sync.dma_start(out=dinvt[:], in_=dinv[:, :])
            dinvft = cpool.tile([P, HALF * P], BF16, tag="dinvft")
            nc.sync.dma_start(out=dinvft[:], in_=dinvf[:, :])
            iota = cpool.tile([P, P], BF16, tag="iota")
            nc.sync.dma_start(out=iota[:], in_=iot[:, :])
            ident = cpool.tile([P, P], BF16, tag="ident")
            nc.sync.dma_start(out=ident[:], in_=idn[:, :])
            idxt = ipool.tile([P, SLOTS // 16], I16, tag="idxt")
            nc.sync.dma_start(out=idxt[:], in_=idx[:, :])
            drelet = ipool.tile([P, SLOTS // P], BF16, tag="drelet")
            nc.sync.dma_start(out=drelet[:], in_=drele[:, :])
            drelot = ipool.tile([P, SLOTS // P], BF16, tag="drelot")
            nc.sync.dma_start(out=drelot[:], in_=drelo[:, :])

            qctr = [0]
            # tile-blocks of window-0 gathers in flight ahead of
            # window-1+processing; needs KPIPE+2 msg pool buffers
            KPIPE = int(os.environ.get("GCN_KPIPE", "2"))

            def gather_stream(plan, msg, table, col_base, win):
                if ablate in ("nogather", "noedge"):
                    return
                pairs = table.rearrange("(a two) d -> a (two d)", two=2)
                for (w, col0, n) in plan["gathers"]:
                    if w != win:
                        continue
                    for s in range(0, n, GMAX):
                        m = min(GMAX, n - s)
                        c = col0 - col_base + s // P
                        nc.gpsimd.dma_gather(
                            out_ap=msg[:, c:c + m // P, :],
                            in_ap=pairs[w * PWROWS:(w + 1) * PWROWS, :],
                            idxs_ap=idxt[:, (col0 * P + s) // 16:
                                         (col0 * P + s + m) // 16],
                            num_idxs=m, num_idxs_reg=m, elem_size=ROWW,
                            queue_num=qctr[0] % NQ)
                        qctr[0] += 1

            def layer_loop(table, process_tb):
                """Window-pipelined tb loop: window-0 gathers run KPIPE
                tile-blocks ahead so a late window-1 AllGather chunk never
                head-of-line-blocks the gpsimd queue."""
                state = {}
                for i in range(n_tb + KPIPE):
                    if i < n_tb:
                        plan = tb_plans[i]
                        col_base = min(c0 for _, c0, _ in plan["gathers"])
                        cols_tb = sum(
                            n for _, _, n in plan["gathers"]) // P
                        msg = mpool.tile([P, cols_tb, ROWW], BF16,
                                         tag="msg")
                        gather_stream(plan, msg, table, col_base, 0)
                        state[i] = (plan, col_base, cols_tb, msg)
                    j = i - KPIPE
                    if 0 <= j < n_tb:
                        plan, col_base, cols_tb, msg = state.pop(j)
                        gather_stream(plan, msg, table, col_base, 1)
                        process_tb(j, plan, col_base, cols_tb, msg)

            def build_st(drelt, plan, col_base, cols_tb, tag):
                st = spool.tile([P, cols_tb, P], BF16, tag=tag)
                nc.vector.tensor_tensor(
                    out=st[:],
                    in0=drelt[:, col_base:col_base + cols_tb, None
                              ].broadcast_to([P, cols_tb, P]),
                    in1=iota[:, None, :].broadcast_to([P, cols_tb, P]),
                    op=mybir.AluOpType.is_equal)
                return st

            HT = HALF  # tiles per half (49)

            def compute_body():
                # ---- phase A: t1_shard = dinv * (x @ W1), compact bf16 ----
                # AllGather each half-shard as soon as it is written so the
                # first collective overlaps the second half's compute.
                with tc.tile_pool(name="xw", bufs=3) as xpool:
                    for c in range(2):
                        for i in range(c * HT, min((c + 1) * HT, NT)):
                            xt = xpool.tile([P, KF, P], BF16, tag="xt")
                            nc.sync.dma_start(
                                out=xt[:],
                                in_=xT.rearrange("(k p) n -> p k n", p=P)[
                                    :, :, i * P:(i + 1) * P])
                            ph = psA.tile([P, D], F32, tag="ph")
                            for k in range(KF):
                                nc.tensor.matmul(
                                    out=ph[:], lhsT=xt[:, k, :],
                                    rhs=w1t[:, k, :],
                                    start=(k == 0), stop=(k == KF - 1))
                            hs = xpool.tile([P, D], BF16, tag="hs")
                            nc.scalar.activation(
                                out=hs[:], in_=ph[:], func=Copy,
                                scale=dinvt[:, i:i + 1])
                            nc.sync.dma_start(
                                out=t1_shard[i * P:(i + 1) * P, :], in_=hs[:])
                        all_gather(t1_shard, t1_full, c)

                # ==================== LAYER 1 ====================
                # t2/AllGather2 for the first half is emitted as soon as the
                # tile-blocks covering tiles 0..HT-1 are aggregated, hiding
                # the collective under the second half's gather stream.
                h1buf = bpool.tile([P, HALF * P], BF16, tag="bigH")
                t2buf = bpool.tile([P, NT, D], BF16, tag="t2buf")

                def finalize_half(c):
                    fp = 64 * c
                    nc.vector.tensor_tensor(
                        out=h1buf[fp:fp + 64, :], in0=h1buf[fp:fp + 64, :],
                        in1=dinvft[fp:fp + 64, :], op=mybir.AluOpType.mult)
                    nc.scalar.activation(
                        out=h1buf[fp:fp + 64, :], in_=h1buf[fp:fp + 64, :],
                        func=mybir.ActivationFunctionType.Relu,
                        bias=b1t[fp:fp + 64, 0:1])
                    for t in range(c * HT, min((c + 1) * HT, NT)):
                        _, fc = fold_slice(t)
                        po = psB.tile([P, D], F32, tag="po")
                        nc.tensor.matmul(
                            out=po[:], lhsT=h1buf[fp:fp + 64, fc:fc + P],
                            rhs=w2t[fp:fp + 64, :], start=True, stop=True)
                        nc.scalar.activation(
                            out=t2buf[:, t, :], in_=po[:], func=Copy,
                            scale=dinvt[:, t:t + 1])
                    lo = c * HT
                    hi = min((c + 1) * HT, NT)
                    nc.sync.dma_start(
                        out=t2_shard.rearrange("(q p) d -> p q d", p=P)[
                            :, lo:hi, :],
                        in_=t2buf[:, lo:hi, :])
                    all_gather(t2_shard, t2_full, c)

                tb_half = (HT + TBS - 1) // TBS - 1   # last tb covering half0

                def process_l1(tb, plan, col_base, cols_tb, msg):
                    ste = build_st(drelet, plan, col_base, cols_tb, "ste")
                    sto = build_st(drelot, plan, col_base, cols_tb, "sto")
                    ntt = len(plan["tiles"])
                    t0_ = plan["tiles"][0][0]
                    selfb = wpool.tile([P, TBS, D], BF16, tag="selfb")
                    nc.sync.dma_start(
                        out=selfb[:, 0:ntt, :],
                        in_=t1_shard[t0_ * P:(t0_ + ntt) * P, :].rearrange(
                            "(q p) d -> p q d", p=P))
                    pt = psB.tile([D, ntt, P], F32, tag="pt")
                    for ti, (t, cols) in enumerate(plan["tiles"]):
                        if ablate in ("noedge", "nomm"):
                            cols = []
                        for ci, col in enumerate(cols):
                            cc = col - col_base
                            nc.tensor.matmul(
                                out=pt[:, ti, :], lhsT=msg[:, cc, 0:D],
                                rhs=ste[:, cc, :],
                                start=(ci == 0), stop=False)
                            nc.tensor.matmul(
                                out=pt[:, ti, :], lhsT=msg[:, cc, D:ROWW],
                                rhs=sto[:, cc, :],
                                start=False, stop=False)
                        # self-loop diagonal term from the local shard
                        nc.tensor.matmul(
                            out=pt[:, ti, :], lhsT=selfb[:, ti, :],
                            rhs=ident[:],
                            start=(len(cols) == 0), stop=True)
                    # one PSUM->SBUF copy per run of tiles on the same fold
                    # half (instead of per tile): far fewer Act-engine ops
                    ti = 0
                    while ti < ntt:
                        t = plan["tiles"][ti][0]
                        fp, fc = fold_slice(t)
                        run = 1
                        while (ti + run < ntt
                               and fold_slice(plan["tiles"][ti + run][0])[0]
                               == fp):
                            run += 1
                        nc.scalar.activation(
                            out=h1buf[fp:fp + 64, fc:fc + run * P],
                            in_=pt[:, ti:ti + run, :], func=Copy)
                        ti += run
                    if tb == tb_half:
                        finalize_half(0)

                layer_loop(t1_full, process_l1)
                finalize_half(1)

                # ==================== LAYER 2 ====================
                obuf = bpool.tile([P, NT, C], F32, tag="bigA")
                mt = cpool.tile([P, NT], F32, tag="mt")
                sums = cpool.tile([P, NT], F32, tag="sums")
                escr = cpool.tile([P, C], F32, tag="escr")
                lst = cpool.tile([P, NT], F32, tag="lst")

                def softmax_half(c):
                    # out[:, lo:hi] = log_softmax(obuf[:, lo:hi] + b2)
                    lo = c * HT
                    hi = min((c + 1) * HT, NT)
                    ob = obuf[:, lo:hi, :]
                    nt = hi - lo
                    # dst-side dinv (moved off the per-tile PSUM drains)
                    nc.vector.tensor_tensor(
                        out=ob, in0=ob,
                        in1=dinvt[:, lo:hi, None].broadcast_to([P, nt, C]),
                        op=mybir.AluOpType.mult)
                    nc.vector.tensor_tensor(
                        out=ob, in0=ob,
                        in1=b2t[:, None, :].broadcast_to([P, nt, C]),
                        op=mybir.AluOpType.add)
                    nc.vector.tensor_reduce(
                        out=mt[:, lo:hi], in_=ob, axis=mybir.AxisListType.X,
                        op=mybir.AluOpType.max)
                    nc.vector.tensor_tensor(
                        out=ob, in0=ob,
                        in1=mt[:, lo:hi, None].broadcast_to([P, nt, C]),
                        op=mybir.AluOpType.subtract)
                    eb = cpool.tile([P, HT, C], F32, tag="ebig")
                    nc.scalar.activation(
                        out=eb[:, 0:nt, :], in_=ob,
                        func=mybir.ActivationFunctionType.Exp)
                    nc.vector.tensor_reduce(
                        out=sums[:, lo:hi], in_=eb[:, 0:nt, :],
                        axis=mybir.AxisListType.X, op=mybir.AluOpType.add)
                    nc.scalar.activation(
                        out=lst[:, lo:hi], in_=sums[:, lo:hi],
                        func=mybir.ActivationFunctionType.Ln)
                    nc.vector.tensor_tensor(
                        out=ob, in0=ob,
                        in1=lst[:, lo:hi, None].broadcast_to([P, nt, C]),
                        op=mybir.AluOpType.subtract)
                    nc.sync.dma_start(out=out[:, lo:hi, :],
                                      in_=obuf[:, lo:hi, :])

                def process_l2(tb, plan, col_base, cols_tb, msg):
                    ste = build_st(drelet, plan, col_base, cols_tb, "ste")
                    sto = build_st(drelot, plan, col_base, cols_tb, "sto")
                    ntt = len(plan["tiles"])
                    t0_ = plan["tiles"][0][0]
                    selfb = wpool.tile([P, TBS, D], BF16, tag="selfb2")
                    nc.sync.dma_start(
                        out=selfb[:, 0:ntt, :],
                        in_=t2_shard[t0_ * P:(t0_ + ntt) * P, :].rearrange(
                            "(q p) d -> p q d", p=P))
                    pa = psB.tile([P, ntt, C], F32, tag="pa")
                    for ti, (t, cols) in enumerate(plan["tiles"]):
                        if ablate in ("noedge", "nomm"):
                            cols = []
                        for ci, col in enumerate(cols):
                            cc = col - col_base
                            nc.tensor.matmul(
                                out=pa[:, ti, :], lhsT=ste[:, cc, :],
                                rhs=msg[:, cc, 0:C],
                                start=(ci == 0), stop=False)
                            nc.tensor.matmul(
                                out=pa[:, ti, :], lhsT=sto[:, cc, :],
                                rhs=msg[:, cc, D:D + C],
                                start=False, stop=False)
                        nc.tensor.matmul(
                            out=pa[:, ti, :], lhsT=ident[:],
                            rhs=selfb[:, ti, 0:C],
                            start=(len(cols) == 0), stop=True)
                    # plain batched PSUM drain; the dst-side dinv scale is
                    # applied on the vector engine in softmax_half instead
                    nc.scalar.activation(
                        out=obuf[:, t0_:t0_ + ntt, :], in_=pa[:, 0:ntt, :],
                        func=Copy)
                    if tb == tb_half:
                        softmax_half(0)

                layer_loop(t2_full, process_l2)
                softmax_half(1)

            for _rep in range(repeat):
                compute_body()

    return nc


# ---------------------------------------------------------------------------
# Entry point
# ---------------------------------------------------------------------------

def prepare(x, edge_index, W1, b1, W2, b2, repeat=1, ablate=None):
    """Build (nc, in_maps, meta) without running — shared by kernel() and
    external benchmarking harnesses."""
    x = np.asarray(x, dtype=np.float32)
    W1 = np.asarray(W1, dtype=np.float32)
    b1 = np.asarray(b1, dtype=np.float32)
    W2 = np.asarray(W2, dtype=np.float32)
    b2 = np.asarray(b2, dtype=np.float32)

    F, D = W1.shape
    C = W2.shape[1]

    meta = _preprocess(x, edge_index)

    nc = build_gcn_module(meta, F, D, C, repeat=repeat, ablate=ablate)
    nc.finalize()

    W2p = np.zeros((P, D), dtype=NPBF16)
    W2p[0:64, :C] = W2.astype(NPBF16)
    W2p[64:128, :C] = W2.astype(NPBF16)
    b1fold = np.empty((P, 1), dtype=np.float32)
    b1fold[0:64, 0] = b1
    b1fold[64:128, 0] = b1
    b2r = np.broadcast_to(b2, (P, C)).astype(np.float32).copy()
    in_maps = []
    for k in range(NCORE):
        in_maps.append({
            "xT": meta["xT"][k],
            "idx": meta["IDX"][k],
            "drele": meta["DRELE"][k],
            "drelo": meta["DRELO"][k],
            "dinv": meta["dinv_pc"][k],
            "dinvf": meta["dinv_fold"][k],
            "W1": W1.astype(NPBF16), "b1f": b1fold, "W2p": W2p, "b2r": b2r,
            "iota": meta["iota"],
            "ident": np.eye(P, dtype=np.float32).astype(NPBF16),
        })
    return nc, in_maps, meta


def kernel(x, edge_index, W1, b1, W2, b2):
    N = np.asarray(x).shape[0]
    C = np.asarray(W2).shape[1]

    nc, in_maps, meta = prepare(x, edge_index, W1, b1, W2, b2)
    NT, SH = meta["NT"], meta["SH"]

    res = run_bass_kernel_spmd(
        nc, in_maps, core_ids=list(range(NCORE)),
        trace=os.environ.get("GCN_TRACE") == "1")
    kernel.last_results = res

    out = np.empty((N, C), dtype=np.float32)
    iof = meta["IOF"]
    for k in range(NCORE):
        ok = np.asarray(res.results[k]["out"]).reshape(P, NT, C)
        pos = iof[k * SH:(k + 1) * SH]
        out[k * SH:(k + 1) * SH] = ok[pos % P, pos // P, :]
    return out
